# revision 15
# baseline (speedup 1.0000x reference)
"""Multi-head causal attention (B=4, T=2048, D=1024, H=16) on 8 trn2 cores.

Sharding: core = (batch b = core//2, head-group g = core%2). Each core gets
x[b] transposed to [D, T], the 8-head slices of Wq/Wk/Wv transposed to
[D, 512], and Wo[:, g*512:(g+1)*512].T = [512, D]. Each core emits a partial
output [T, D]; host sums the two group partials per batch and adds bo.

Device kernel (per core), all activations kept with the contraction dim on
partitions so no on-device transposes are needed:
  qT/kT [512, T] = W.T @ xT   (head h occupies rows h*64..h*64+64; head pair
                               2c/2c+1 shares 128-partition chunk c)
  v     [T, 512] natural, stored per head-pair in 192-wide windows:
        [v_A(64) | onesA@64 | onesB@96 | v_B(128:192)]
  scoresT[tk, tq] per head = kT_chunk.T @ qT_block, head pairs run as
        concurrent row-packed matmuls (K=64 at array rows 0-63 / 64-127)
  expT = exp(scoresT/8) via ACT psum->sbuf (bf16), causal mask via gpsimd
        memset (full rects) + affine_select (diagonal triangles)
  ctxT+denoms = [v|ones] windows (M=128) @ expT: ctx_A at psum rows 0-63 with
        den_A at row 64; ctx_B at rows 64-127 with den_B at row 32
  normalize: reciprocal of den rows, K=1 broadcast matmuls to all partitions,
        DVE multiply into ctxT [512, T]
  out[T, D] partial = ctxT_chunk.T @ WoT, DMA'd straight from PSUM.
"""

import numpy as np
from contextlib import ExitStack

import concourse.bass as bass
import concourse.mybir as mybir
import concourse.tile as tile
from concourse.bass_utils import run_bass_kernel_spmd

F32 = mybir.dt.float32
F32R = mybir.dt.float32r
BF16 = mybir.dt.bfloat16
AF = mybir.ActivationFunctionType

P = 128


def _legalize_single_wait(nc):
    """Split multi-wait instructions into single-wait form.

    The walrus build in this container rejects any instruction carrying more
    than one sync wait ("Too many sync wait commands" in setupSyncWait). The
    Tile scheduler freely attaches several sem waits to one instruction, so
    post-process the serialized BIR: for each instruction with N>1 waits,
    insert N-1 NoOps on the same engine immediately before it, each carrying
    one of the waits. Per-engine program order makes this semantically
    identical (the engine's dispatch blocks on each wait in sequence).
    """
    import orjson

    data = orjson.loads(mybir.module_to_json_bytes(nc.m))
    ctr = 0
    for fn in data["functions"]:
        for bb in fn["blocks"]:
            out = []
            for ins in bb["instructions"]:
                si = ins.get("sync_info")
                waits = (si or {}).get("on_wait") or []
                if len(waits) > 1:
                    for w in waits[:-1]:
                        ctr += 1
                        out.append({
                            "engine": ins["engine"],
                            "ins": [],
                            "outs": [],
                            "name": f"I-sw{ctr}",
                            "opcode": "NoOp",
                            "sync_info": {"on_update": [], "on_wait": [w]},
                        })
                    si["on_wait"] = [waits[-1]]
                out.append(ins)
            bb["instructions"] = out
    nc.m = mybir.parse(data)
    return ctr


def build_mha_nc(T=2048, D=1024, DG=512, legalize=True):
    """One-core kernel: full inputs for one (batch, head-group)."""
    HD = 64
    KC = D // P            # contraction chunks for projections (8)
    MC = DG // P           # head-pair chunks (4)
    TB = T // 512          # 512-wide t blocks (4)
    NT = T // P            # 128-wide t chunks (16)
    scale = 1.0 / np.sqrt(HD)

    nc = bass.Bass()
    xT_d = nc.declare_dram_parameter("xT", [D, T], F32R, isOutput=False)
    wq_d = nc.declare_dram_parameter("wqT", [D, DG], F32R, isOutput=False)
    wk_d = nc.declare_dram_parameter("wkT", [D, DG], F32R, isOutput=False)
    wv_d = nc.declare_dram_parameter("wvT", [D, DG], F32R, isOutput=False)
    wo_d = nc.declare_dram_parameter("woT", [DG, D], F32R, isOutput=False)
    out_d = nc.declare_dram_parameter("out", [T, D], F32, isOutput=True)

    with tile.TileContext(nc) as tc, ExitStack() as ctx:
        persist = ctx.enter_context(tc.tile_pool(name="persist", bufs=1))
        qT = persist.tile([P, MC, T], F32R, tag="qT")
        kT = persist.tile([P, MC, T], F32R, tag="kT")
        vext = persist.tile([P, NT, MC, 192], BF16, tag="vext")
        ctxT = persist.tile([P, MC, T], F32R, tag="ctxT")
        # den rows 0 (head A) and 32 (head B) receive reciprocals; partition
        # bases must be 32-aligned, so the broadcast matmul contracts over 64
        # rows with a selector that zeroes the unused ones.
        den = persist.tile([64, 512], F32R, tag="den")
        sel = persist.tile([64, P], F32R, tag="sel")
        stage_f32 = persist.tile([64, 512], F32, tag="stage_f32")

        # ones columns for the denominator rows; selector rows for broadcast.
        # pad columns 65:96 / 97:128 are read as stationary junk (their output
        # rows are never consumed) but must be initialized. memset can't write
        # fp32r directly, so f32 staging + copy.
        nc.gpsimd.memset(vext[:, :, :, 64:96], 0.0)
        nc.gpsimd.memset(vext[:, :, :, 96:128], 0.0)
        nc.gpsimd.memset(vext[:, :, :, 64:65], 1.0)
        nc.gpsimd.memset(vext[:, :, :, 96:97], 1.0)
        nc.gpsimd.memset(stage_f32[:], 0.0)
        nc.vector.tensor_copy(den[:], stage_f32[:])
        nc.gpsimd.memset(stage_f32[0:1, 0:64], 1.0)
        nc.gpsimd.memset(stage_f32[32:33, 64:P], 1.0)
        nc.vector.tensor_copy(sel[:], stage_f32[:, 0:P])

        # ---------------- phase 1: QKV projections ----------------
        with tc.tile_pool(name="wqkv", bufs=1) as wpool, \
             tc.tile_pool(name="xin", bufs=2) as xpool, \
             tc.tile_pool(name="ps_qkv", bufs=2, space="PSUM") as pq:
            wq_s = wpool.tile([P, KC, DG], F32R, tag="wq")
            wk_s = wpool.tile([P, KC, DG], F32R, tag="wk")
            wv_s = wpool.tile([P, KC, DG], F32R, tag="wv")
            nc.sync.dma_start(wq_s[:], wq_d[:].rearrange("(o p) m -> p o m", p=P))
            nc.sync.dma_start(wk_s[:], wk_d[:].rearrange("(o p) m -> p o m", p=P))
            nc.sync.dma_start(wv_s[:], wv_d[:].rearrange("(o p) m -> p o m", p=P))
            xT_r = xT_d[:].rearrange("(o p) t -> p o t", p=P)

            for tb in range(TB):
                tsl = slice(tb * 512, (tb + 1) * 512)
                xt = xpool.tile([P, KC, 512], F32R, tag="x")
                nc.sync.dma_start(xt[:], xT_r[:, :, tsl])
                for c in range(MC):
                    ps_q = pq.tile([P, 512], F32, tag="q")
                    ps_k = pq.tile([P, 512], F32, tag="k")
                    for k in range(KC):
                        nc.tensor.matmul(ps_q[:], wq_s[:, k, c * P:(c + 1) * P],
                                         xt[:, k, :], start=(k == 0), stop=(k == KC - 1))
                    for k in range(KC):
                        nc.tensor.matmul(ps_k[:], wk_s[:, k, c * P:(c + 1) * P],
                                         xt[:, k, :], start=(k == 0), stop=(k == KC - 1))
                    nc.vector.tensor_copy(qT[:, c, tsl], ps_q[:])
                    nc.vector.tensor_copy(kT[:, c, tsl], ps_k[:])
                for tcl in range(4):
                    tg = tb * 4 + tcl
                    ps_v = pq.tile([P, DG], F32, tag="v")
                    for k in range(KC):
                        nc.tensor.matmul(ps_v[:], xt[:, k, tcl * P:(tcl + 1) * P],
                                         wv_s[:, k, :], start=(k == 0), stop=(k == KC - 1))
                    pv = ps_v.rearrange("p (c two e) -> p c two e", two=2, e=HD)
                    nc.vector.tensor_copy(vext[:, tg, :, 0:HD], pv[:, :, 0, :])
                    nc.vector.tensor_copy(vext[:, tg, :, P:P + HD], pv[:, :, 1, :])

        # ---------------- phase 2: attention ----------------
        with tc.tile_pool(name="exp", bufs=2) as epool, \
             tc.tile_pool(name="bc_sb", bufs=2) as bcpool, \
             tc.tile_pool(name="ps_s", bufs=2, space="PSUM") as pspool, \
             tc.tile_pool(name="ps_ctx", bufs=1, space="PSUM") as pcpool:
            for j in range(TB):
                nch = 4 * (j + 1)          # tk chunks needed for this tq block
                jsl = slice(j * 512, (j + 1) * 512)
                for c in range(MC):
                    expA = epool.tile([P, TB * 4, 512], BF16, tag="expA")
                    expB = epool.tile([P, TB * 4, 512], BF16, tag="expB")
                    ps_cA = pcpool.tile([P, 512], F32, tag="cA")
                    ps_cB = pcpool.tile([P, 512], F32, tag="cB")
                    ps_list = []
                    for ck in range(nch):
                        ksl = slice(ck * P, (ck + 1) * P)
                        ps_sA = pspool.tile([P, 512], F32, tag="sA")
                        ps_sB = pspool.tile([P, 512], F32, tag="sB")
                        nc.tensor.matmul(ps_sA[:], kT[0:64, c, ksl],
                                         qT[0:64, c, jsl], start=True, stop=True)
                        nc.tensor.matmul(ps_sB[:], kT[64:P, c, ksl],
                                         qT[64:P, c, jsl], start=True, stop=True)
                        off = ck * P - j * 512
                        if off >= 0:
                            # diagonal chunk: exp only the live columns,
                            # zero the dead rect, mask the triangle
                            for exp_t, ps_t in ((expA, ps_sA), (expB, ps_sB)):
                                if off > 0:
                                    nc.gpsimd.memset(exp_t[:, ck, 0:off], 0.0)
                                nc.scalar.activation(exp_t[:, ck, off:512],
                                                     ps_t[:, off:512], AF.Exp, scale=scale)
                                nc.gpsimd.affine_select(
                                    out=exp_t[:, ck, off:off + P],
                                    in_=exp_t[:, ck, off:off + P],
                                    compare_op=mybir.AluOpType.is_ge,
                                    fill=0.0, base=0,
                                    pattern=[[1, P]], channel_multiplier=-1)
                        else:
                            nc.scalar.activation(expA[:, ck, :], ps_sA[:], AF.Exp, scale=scale)
                            nc.scalar.activation(expB[:, ck, :], ps_sB[:], AF.Exp, scale=scale)
                        # software-pipeline the ctx matmuls one chunk behind
                        ps_list.append(ck)
                        if len(ps_list) >= 2 or ck == nch - 1:
                            for cck in ps_list if ck == nch - 1 else ps_list[:1]:
                                nc.tensor.matmul(ps_cA[:], vext[:, cck, c, 0:P],
                                                 expA[:, cck, :],
                                                 start=(cck == 0), stop=(cck == nch - 1))
                                nc.tensor.matmul(ps_cB[:], vext[:, cck, c, 64:192],
                                                 expB[:, cck, :],
                                                 start=(cck == 0), stop=(cck == nch - 1))
                            ps_list = ps_list[1:] if ck != nch - 1 else []
                    # denominators -> reciprocals -> broadcast -> normalize.
                    # den_A sits at ps_cA row 64, den_B at ps_cB row 32;
                    # 1-input DVE ops may write cross-partition-base, so both
                    # land in den rows 0/1, feeding one K=2 broadcast matmul.
                    with nc.allow_low_precision(reason="denominator reciprocal feeds an fp32r matmul; fp32r rounding is the accepted precision here"):
                        nc.vector.reciprocal(den[0:1, :], ps_cA[64:65, :])
                        nc.vector.reciprocal(den[32:33, :], ps_cB[32:33, :])
                    ps_bc = pspool.tile([P, 512], F32, tag="sA")
                    nc.tensor.matmul(ps_bc[:], sel[:], den[:], start=True, stop=True)
                    bc_sb = bcpool.tile([P, 512], F32, tag="bc")
                    nc.vector.tensor_copy(bc_sb[:], ps_bc[:])
                    nc.vector.tensor_mul(ctxT[0:64, c, jsl], ps_cA[0:64, :], bc_sb[0:64, :])
                    nc.vector.tensor_mul(ctxT[64:P, c, jsl], ps_cB[64:P, :], bc_sb[64:P, :])

        # ---------------- phase 3: output projection ----------------
        with tc.tile_pool(name="wo", bufs=1) as wopool, \
             tc.tile_pool(name="o_sb", bufs=3) as ospool, \
             tc.tile_pool(name="ps_out", bufs=4, space="PSUM") as popool:
            wo_s = wopool.tile([P, MC, D], F32R, tag="wo")
            nc.sync.dma_start(wo_s[:], wo_d[:].rearrange("(o p) m -> p o m", p=P))
            for tcix in range(NT):
                for n in range(D // 512):
                    ps_o = popool.tile([P, 512], F32, tag="o")
                    for kc in range(MC):
                        nc.tensor.matmul(ps_o[:], ctxT[:, kc, tcix * P:(tcix + 1) * P],
                                         wo_s[:, kc, n * 512:(n + 1) * 512],
                                         start=(kc == 0), stop=(kc == MC - 1))
                    o_sb = ospool.tile([P, 512], F32, tag="o_sb")
                    nc.vector.tensor_copy(o_sb[:], ps_o[:])
                    nc.sync.dma_start(out_d[tcix * P:(tcix + 1) * P, n * 512:(n + 1) * 512],
                                      o_sb[:])
    if legalize:
        _legalize_single_wait(nc)
    return nc


_NC_CACHE = {}


def _get_nc(T=2048, D=1024, DG=512):
    key = (T, D, DG)
    if key not in _NC_CACHE:
        _NC_CACHE[key] = build_mha_nc(T, D, DG)
    return _NC_CACHE[key]


def make_in_maps(x, Wq, Wk, Wv, Wo):
    """Shard full inputs into per-core input maps (8 cores)."""
    x = np.asarray(x, dtype=np.float32)
    in_maps = []
    for core in range(8):
        b, g = core // 2, core % 2
        gs = slice(g * 512, (g + 1) * 512)
        in_maps.append({
            "xT": np.ascontiguousarray(x[b].T),
            "wqT": np.ascontiguousarray(np.asarray(Wq, np.float32)[gs, :].T),
            "wkT": np.ascontiguousarray(np.asarray(Wk, np.float32)[gs, :].T),
            "wvT": np.ascontiguousarray(np.asarray(Wv, np.float32)[gs, :].T),
            "woT": np.ascontiguousarray(np.asarray(Wo, np.float32)[:, gs].T),
        })
    return in_maps


def combine_outputs(results, bo):
    bo = np.asarray(bo, dtype=np.float32)
    outs = []
    for b in range(4):
        outs.append(results[2 * b]["out"] + results[2 * b + 1]["out"] + bo[None, :])
    return np.stack(outs, axis=0).astype(np.float32)


def run(x, Wq, Wk, Wv, Wo, bo, trace=False):
    nc = _get_nc(x.shape[1], x.shape[2], 512)
    in_maps = make_in_maps(x, Wq, Wk, Wv, Wo)
    res = run_bass_kernel_spmd(nc, in_maps, list(range(8)), trace=trace)
    return combine_outputs(res.results, bo), res


def kernel(x, Wq, Wk, Wv, Wo, bo):
    out, _ = run(x, Wq, Wk, Wv, Wo, bo)
    return out


# revision 30
# speedup vs baseline: 5.3581x; 5.3581x over previous
"""Multi-head causal attention (B=4, T=2048, D=1024, H=16) on 8 trn2 cores.

Sharding: core = (batch b = core//2, head-group g = core%2). Each core gets
x[b] transposed to [D, T], the 8-head slices of Wq/Wk/Wv transposed to
[D, 512], and Wo[:, g*512:(g+1)*512].T = [512, D]. Each core emits a partial
output [T, D]; host sums the two group partials per batch and adds bo.

Device kernel (per core), all activations kept with the contraction dim on
partitions so no on-device transposes are needed:
  qT/kT [512, T] = W.T @ xT   (head h occupies rows h*64..h*64+64; head pair
                               2c/2c+1 shares 128-partition chunk c)
  v     [T, 512] natural, stored per head-pair in 176-wide windows:
        [v_A(0:64) | onesA@64 | onesB@80 | v_B(112:176)]; window_A = cols
        [0:128], window_B = cols [48:176] (psum out row = col within window)
  scoresT[tk, tq] per head = kT_chunk.T @ qT_block, head pairs run as
        concurrent row-packed matmuls (K=64 at array rows 0-63 / 64-127)
  expT = exp(scoresT/8) via ACT psum->sbuf (bf16), causal mask via gpsimd
        memset (full rects) + affine_select (diagonal triangles)
  ctxT+denoms = [v|ones] windows (M=128) @ expT: ctx_A at psum rows 0-63 with
        den_A at row 64; ctx_B at rows 64-127 with den_B at row 32
  normalize: reciprocals into den rows 0/32, one K=64 selector matmul
        broadcasts them to all partitions, DVE multiply into ctxT [512, T]
  out[T, D] partial = ctxT_chunk.T @ WoT per tq block, interleaved with the
        attention phase via pre-allocated disjoint PSUM banks.

All matmuls run as float32r (1 cyc/row at N>=256); exp/v/ctx-matmul inputs
are bf16. Verified on 8 trn2 cores: relative error 2.2e-3 vs the fp32 jax
reference; cost-model timeline ~324 us/core.
"""

import numpy as np
from contextlib import ExitStack

import concourse.bass as bass
import concourse.mybir as mybir
import concourse.tile as tile
from concourse.bass_utils import run_bass_kernel_spmd

F32 = mybir.dt.float32
F32R = mybir.dt.float32r
BF16 = mybir.dt.bfloat16
AF = mybir.ActivationFunctionType

P = 128


def _legalize_single_wait(nc):
    """Split multi-wait instructions into single-wait form.

    The walrus build in this container rejects any instruction carrying more
    than one sync wait ("Too many sync wait commands" in setupSyncWait). The
    Tile scheduler freely attaches several sem waits to one instruction, so
    post-process the serialized BIR: for each instruction with N>1 waits,
    insert N-1 NoOps on the same engine immediately before it, each carrying
    one of the waits. Per-engine program order makes this semantically
    identical (the engine's dispatch blocks on each wait in sequence).
    """
    import orjson

    data = orjson.loads(mybir.module_to_json_bytes(nc.m))
    ctr = 0
    for fn in data["functions"]:
        for bb in fn["blocks"]:
            out = []
            for ins in bb["instructions"]:
                si = ins.get("sync_info")
                waits = (si or {}).get("on_wait") or []
                if len(waits) > 1:
                    for w in waits[:-1]:
                        ctr += 1
                        out.append({
                            "engine": ins["engine"],
                            "ins": [],
                            "outs": [],
                            "name": f"I-sw{ctr}",
                            "opcode": "NoOp",
                            "sync_info": {"on_update": [], "on_wait": [w]},
                        })
                    si["on_wait"] = [waits[-1]]
                out.append(ins)
            bb["instructions"] = out
    nc.m = mybir.parse(data)
    return ctr


def build_mha_nc(T=2048, D=1024, DG=512, legalize=True, exp_group=2, s_bufs=2, ctx_bufs=2):
    """One-core kernel: full inputs for one (batch, head-group)."""
    HD = 64
    KC = D // P            # contraction chunks for projections (8)
    MC = DG // P           # head-pair chunks (4)
    TB = T // 512          # 512-wide t blocks (4)
    NT = T // P            # 128-wide t chunks (16)
    scale = 1.0 / np.sqrt(HD)

    nc = bass.Bass()
    xT_d = nc.declare_dram_parameter("xT", [D, T], F32R, isOutput=False)
    wq_d = nc.declare_dram_parameter("wqT", [D, DG], F32R, isOutput=False)
    wk_d = nc.declare_dram_parameter("wkT", [D, DG], F32R, isOutput=False)
    wv_d = nc.declare_dram_parameter("wvT", [D, DG], F32R, isOutput=False)
    wo_d = nc.declare_dram_parameter("woT", [DG, D], F32R, isOutput=False)
    out_d = nc.declare_dram_parameter("out", [T, D], F32, isOutput=True)

    with tile.TileContext(nc) as tc, ExitStack() as ctx:
        persist = ctx.enter_context(tc.tile_pool(name="persist", bufs=1))
        qT = persist.tile([P, MC, T], F32R, tag="qT")
        kT = persist.tile([P, MC, T], F32R, tag="kT")
        vext = persist.tile([P, NT, MC, 176], BF16, tag="vext")
        ctxT = persist.tile([P, MC, T], F32R, tag="ctxT")
        # den rows 0 (head A) and 32 (head B) receive reciprocals; partition
        # bases must be 32-aligned, so the broadcast matmul contracts over 64
        # rows with a selector that zeroes the unused ones.
        den = persist.tile([64, 512], F32R, tag="den")
        sel = persist.tile([64, P], F32R, tag="sel")
        stage_pool = tc.alloc_tile_pool(name="stage", bufs=1)
        stage_f32 = stage_pool.tile([64, 512], F32, tag="stage_f32")

        # ones columns for the denominator rows; selector rows for broadcast.
        # pad columns 65:96 / 97:128 are read as stationary junk (their output
        # rows are never consumed) but must be initialized. memset can't write
        # fp32r directly, so f32 staging + copy.
        nc.gpsimd.memset(vext[:, :, :, 64:112], 0.0)
        nc.gpsimd.memset(vext[:, :, :, 64:65], 1.0)
        nc.gpsimd.memset(vext[:, :, :, 80:81], 1.0)
        nc.gpsimd.memset(stage_f32[:], 0.0)
        nc.vector.tensor_copy(den[:], stage_f32[:])
        nc.gpsimd.memset(stage_f32[0:1, 0:64], 1.0)
        nc.gpsimd.memset(stage_f32[32:33, 64:P], 1.0)
        nc.vector.tensor_copy(sel[:], stage_f32[:, 0:P])
        stage_pool.release()

        # out-projection pools pre-allocated (disjoint PSUM banks / SBUF) so
        # the per-block output projection overlaps the attention phase
        ps_out = ctx.enter_context(tc.tile_pool(name="ps_out", bufs=2, space="PSUM"))
        ospool = ctx.enter_context(tc.tile_pool(name="o_sb", bufs=2))

        # ---------------- phase 1: QKV projections ----------------
        with tc.tile_pool(name="wqkv", bufs=1) as wpool, \
             tc.tile_pool(name="xin", bufs=2) as xpool, \
             tc.tile_pool(name="ps_qkv", bufs=2, space="PSUM") as pq, \
             tc.tile_pool(name="ps_v", bufs=2, space="PSUM") as pvpool:
            wq_s = wpool.tile([P, KC, DG], F32R, tag="wq")
            wk_s = wpool.tile([P, KC, DG], F32R, tag="wk")
            wv_s = wpool.tile([P, KC, DG], F32R, tag="wv")
            # per-k-chunk DMAs so the first matmuls only wait for their own
            # 256KB slice, not the whole 2MB weight load. x block 0 is issued
            # before the weights so the first matmul chain starts ASAP.
            wq_r = wq_d[:].rearrange("(o p) m -> p o m", p=P)
            wk_r = wk_d[:].rearrange("(o p) m -> p o m", p=P)
            wv_r = wv_d[:].rearrange("(o p) m -> p o m", p=P)
            xT_r = xT_d[:].rearrange("(o p) t -> p o t", p=P)
            xt0 = xpool.tile([P, KC, 512], F32R, tag="x", name="xt0")
            for k in range(KC):
                nc.sync.dma_start(xt0[:, k, :], xT_r[:, k, 0:512])
                nc.sync.dma_start(wq_s[:, k, :], wq_r[:, k, :])
                nc.sync.dma_start(wk_s[:, k, :], wk_r[:, k, :])
                nc.sync.dma_start(wv_s[:, k, :], wv_r[:, k, :])

            for tb in range(TB):
                tsl = slice(tb * 512, (tb + 1) * 512)
                if tb == 0:
                    xt = xt0
                else:
                    xt = xpool.tile([P, KC, 512], F32R, tag="x")
                    for k in range(KC):
                        nc.sync.dma_start(xt[:, k, :], xT_r[:, k, tsl])
                for c in range(MC):
                    ps_q = pq.tile([P, 512], F32, tag="q")
                    ps_k = pq.tile([P, 512], F32, tag="k")
                    for k in range(KC):
                        nc.tensor.matmul(ps_q[:], wq_s[:, k, c * P:(c + 1) * P],
                                         xt[:, k, :], start=(k == 0), stop=(k == KC - 1))
                    for k in range(KC):
                        nc.tensor.matmul(ps_k[:], wk_s[:, k, c * P:(c + 1) * P],
                                         xt[:, k, :], start=(k == 0), stop=(k == KC - 1))
                    nc.vector.tensor_copy(qT[:, c, tsl], ps_q[:])
                    nc.vector.tensor_copy(kT[:, c, tsl], ps_k[:])
                for tcl in range(4):
                    tg = tb * 4 + tcl
                    ps_v = pvpool.tile([P, DG], F32, tag="v")
                    for k in range(KC):
                        nc.tensor.matmul(ps_v[:], xt[:, k, tcl * P:(tcl + 1) * P],
                                         wv_s[:, k, :], start=(k == 0), stop=(k == KC - 1))
                    pv = ps_v.rearrange("p (c two e) -> p c two e", two=2, e=HD)
                    nc.vector.tensor_copy(vext[:, tg, :, 0:HD], pv[:, :, 0, :])
                    nc.vector.tensor_copy(vext[:, tg, :, 112:112 + HD], pv[:, :, 1, :])

        # ---------------- phases 2+3: attention, then output projection.
        # The wo pool opens first so its DMA overlaps the attention phase.
        wopool = ctx.enter_context(tc.tile_pool(name="wo", bufs=1))
        wo_s = wopool.tile([P, MC, D], F32R, tag="wo")
        wo_r = wo_d[:].rearrange("(o p) m -> p o m", p=P)
        for kc in range(MC):
            nc.sync.dma_start(wo_s[:, kc, :], wo_r[:, kc, :])

        # ---------------- phase 2: attention ----------------
        # PSUM: scores tag "s" [128,2,512] (2 banks) x bufs 2 = 4 banks,
        # ctx accumulators cA/cB x bufs 2 = 4 banks; the broadcast matmul
        # borrows a "s" slot. Total exactly 8 banks.
        with tc.tile_pool(name="exp", bufs=2) as epool, \
             tc.tile_pool(name="bc_sb", bufs=1) as bcpool, \
             tc.tile_pool(name="ps_s", bufs=s_bufs, space="PSUM") as pspool, \
             tc.tile_pool(name="ps_ctx", bufs=ctx_bufs, space="PSUM") as pcpool:
            for j in range(TB):
                nch = 4 * (j + 1)          # tk chunks needed for this tq block
                jsl = slice(j * 512, (j + 1) * 512)
                for c in range(MC):
                    expA = epool.tile([P, TB * 4, 512], BF16, tag="expA")
                    expB = epool.tile([P, TB * 4, 512], BF16, tag="expB")
                    ps_cA = pcpool.tile([P, 512], F32, tag="cA")
                    ps_cB = pcpool.tile([P, 512], F32, tag="cB")

                    def emit_ctx2(g2, G, c=c, expA=expA, expB=expB,
                                  ps_cA=ps_cA, ps_cB=ps_cB, nch=nch):
                        for cck in range(G * g2, G * g2 + G):
                            nc.tensor.matmul(ps_cA[:], vext[:, cck, c, 0:P],
                                             expA[:, cck, :],
                                             start=(cck == 0), stop=(cck == nch - 1))
                            nc.tensor.matmul(ps_cB[:], vext[:, cck, c, 48:176],
                                             expB[:, cck, :],
                                             start=(cck == 0), stop=(cck == nch - 1))

                    # exp_group-chunk groups: one scores psum tile per head
                    # per group, one exp instruction per head for non-diag
                    # groups. ctx matmuls run one group behind.
                    G = exp_group
                    for g in range(nch // G):
                        ck0 = G * g
                        ps_sA = pspool.tile([P, G, 512], F32, tag="s")
                        ps_sB = pspool.tile([P, G, 512], F32, tag="s")
                        for i in range(G):
                            ksl = slice((ck0 + i) * P, (ck0 + i + 1) * P)
                            nc.tensor.matmul(ps_sA[:, i, :], kT[0:64, c, ksl],
                                             qT[0:64, c, jsl], start=True, stop=True)
                            nc.tensor.matmul(ps_sB[:, i, :], kT[64:P, c, ksl],
                                             qT[64:P, c, jsl], start=True, stop=True)
                        if (ck0 + G - 1) * P - j * 512 >= 0:
                            # group overlaps the diagonal: per-chunk ranged exp
                            for i in range(G):
                                ck = ck0 + i
                                off = max(ck * P - j * 512, 0)
                                for exp_t, ps_t in ((expA, ps_sA), (expB, ps_sB)):
                                    if off > 0:
                                        nc.gpsimd.memset(exp_t[:, ck, 0:off], 0.0)
                                    nc.scalar.activation(exp_t[:, ck, off:512],
                                                         ps_t[:, i, off:512],
                                                         AF.Exp, scale=scale)
                                    if off < 512:
                                        nc.gpsimd.affine_select(
                                            out=exp_t[:, ck, off:off + P],
                                            in_=exp_t[:, ck, off:off + P],
                                            compare_op=mybir.AluOpType.is_ge,
                                            fill=0.0, base=0,
                                            pattern=[[1, P]], channel_multiplier=-1)
                        else:
                            nc.scalar.activation(expA[:, ck0:ck0 + G, :], ps_sA[:],
                                                 AF.Exp, scale=scale)
                            nc.scalar.activation(expB[:, ck0:ck0 + G, :], ps_sB[:],
                                                 AF.Exp, scale=scale)
                        if g >= 1:
                            emit_ctx2(g - 1, G)
                    emit_ctx2(nch // G - 1, G)
                    # denominators -> reciprocals -> broadcast -> normalize.
                    # den_A sits at ps_cA row 64, den_B at ps_cB row 32;
                    # 1-input DVE ops may write cross-partition-base, so both
                    # land in den rows 0/1, feeding one K=2 broadcast matmul.
                    with nc.allow_low_precision(reason="denominator reciprocal feeds an fp32r matmul; fp32r rounding is the accepted precision here"):
                        nc.vector.reciprocal(den[0:1, :], ps_cA[64:65, :])
                        nc.vector.reciprocal(den[32:33, :], ps_cB[32:33, :])
                    ps_bc_t = pspool.tile([P, exp_group, 512], F32, tag="s", name="ps_bc")
                    ps_bc = ps_bc_t[:, 0, :]
                    nc.tensor.matmul(ps_bc[:], sel[:], den[:], start=True, stop=True)
                    bc_sb = bcpool.tile([P, 512], F32, tag="bc")
                    nc.vector.tensor_copy(bc_sb[:], ps_bc[:])
                    nc.vector.tensor_mul(ctxT[0:64, c, jsl], ps_cA[0:64, :], bc_sb[0:64, :])
                    nc.vector.tensor_mul(ctxT[64:P, c, jsl], ps_cB[64:P, :], bc_sb[64:P, :])

                # output projection for this tq block — overlaps the next
                # block's attention (pre-allocated disjoint pools)
                for tcix in range(4 * j, 4 * j + 4):
                    for n in range(D // 512):
                        ps_o = ps_out.tile([P, 512], F32, tag="o")
                        for kc in range(MC):
                            nc.tensor.matmul(ps_o[:], ctxT[:, kc, tcix * P:(tcix + 1) * P],
                                             wo_s[:, kc, n * 512:(n + 1) * 512],
                                             start=(kc == 0), stop=(kc == MC - 1))
                        o_sb = ospool.tile([P, 512], F32, tag="o_sb")
                        nc.vector.tensor_copy(o_sb[:], ps_o[:])
                        nc.sync.dma_start(
                            out_d[tcix * P:(tcix + 1) * P, n * 512:(n + 1) * 512],
                            o_sb[:])
    if legalize:
        _legalize_single_wait(nc)
    return nc


_NC_CACHE = {}


def _get_nc(T=2048, D=1024, DG=512):
    key = (T, D, DG)
    if key not in _NC_CACHE:
        _NC_CACHE[key] = build_mha_nc(T, D, DG)
    return _NC_CACHE[key]


def make_in_maps(x, Wq, Wk, Wv, Wo):
    """Shard full inputs into per-core input maps (8 cores)."""
    x = np.asarray(x, dtype=np.float32)
    in_maps = []
    for core in range(8):
        b, g = core // 2, core % 2
        gs = slice(g * 512, (g + 1) * 512)
        in_maps.append({
            "xT": np.ascontiguousarray(x[b].T),
            "wqT": np.ascontiguousarray(np.asarray(Wq, np.float32)[gs, :].T),
            "wkT": np.ascontiguousarray(np.asarray(Wk, np.float32)[gs, :].T),
            "wvT": np.ascontiguousarray(np.asarray(Wv, np.float32)[gs, :].T),
            "woT": np.ascontiguousarray(np.asarray(Wo, np.float32)[:, gs].T),
        })
    return in_maps


def combine_outputs(results, bo):
    bo = np.asarray(bo, dtype=np.float32)
    outs = []
    for b in range(4):
        outs.append(results[2 * b]["out"] + results[2 * b + 1]["out"] + bo[None, :])
    return np.stack(outs, axis=0).astype(np.float32)


def run(x, Wq, Wk, Wv, Wo, bo, trace=False):
    nc = _get_nc(x.shape[1], x.shape[2], 512)
    in_maps = make_in_maps(x, Wq, Wk, Wv, Wo)
    res = run_bass_kernel_spmd(nc, in_maps, list(range(8)), trace=trace)
    return combine_outputs(res.results, bo), res


def kernel(x, Wq, Wk, Wv, Wo, bo):
    out, _ = run(x, Wq, Wk, Wv, Wo, bo)
    return out


# revision 34
# speedup vs baseline: 5.4020x; 1.0082x over previous
"""Multi-head causal attention (B=4, T=2048, D=1024, H=16) on 8 trn2 cores.

Sharding: core = (batch b = core//2, head-group g = core%2). Each core gets
x[b] transposed to [D, T], the 8-head slices of Wq/Wk/Wv transposed to
[D, 512], and Wo[:, g*512:(g+1)*512].T = [512, D]. Each core emits a partial
output [T, D]; host sums the two group partials per batch and adds bo.

Device kernel (per core), all activations kept with the contraction dim on
partitions so no on-device transposes are needed:
  qT/kT [512, T] = W.T @ xT   (head h occupies rows h*64..h*64+64; head pair
                               2c/2c+1 shares 128-partition chunk c)
  v     [T, 512] natural, stored per head-pair in 176-wide windows:
        [v_A(0:64) | onesA@64 | onesB@80 | v_B(112:176)]; window_A = cols
        [0:128], window_B = cols [48:176] (psum out row = col within window)
  scoresT[tk, tq] per head = kT_chunk.T @ qT_block, head pairs run as
        concurrent row-packed matmuls (K=64 at array rows 0-63 / 64-127)
  expT = exp(scoresT/8) via ACT psum->sbuf (bf16), causal mask via gpsimd
        memset (full rects) + affine_select (diagonal triangles)
  ctxT+denoms = [v|ones] windows (M=128) @ expT: ctx_A at psum rows 0-63 with
        den_A at row 64; ctx_B at rows 64-127 with den_B at row 32
  normalize: reciprocals into den rows 0/32, one K=64 selector matmul
        broadcasts them to all partitions, DVE multiply into ctxT [512, T]
  out[T, D] partial = ctxT_chunk.T @ WoT per tq block, interleaved with the
        attention phase via pre-allocated disjoint PSUM banks.

All matmuls run as float32r (1 cyc/row at N>=256); exp/v/ctx-matmul inputs
are bf16. Verified on 8 trn2 cores: relative error 2.2e-3 vs the fp32 jax
reference; cost-model timeline ~324 us/core.
"""

import numpy as np
from contextlib import ExitStack

import concourse.bass as bass
import concourse.mybir as mybir
import concourse.tile as tile
from concourse.bass_utils import run_bass_kernel_spmd

F32 = mybir.dt.float32
F32R = mybir.dt.float32r
BF16 = mybir.dt.bfloat16
AF = mybir.ActivationFunctionType

P = 128


def _legalize_single_wait(nc):
    """Split multi-wait instructions into single-wait form.

    The walrus build in this container rejects any instruction carrying more
    than one sync wait ("Too many sync wait commands" in setupSyncWait). The
    Tile scheduler freely attaches several sem waits to one instruction, so
    post-process the serialized BIR: for each instruction with N>1 waits,
    insert N-1 NoOps on the same engine immediately before it, each carrying
    one of the waits. Per-engine program order makes this semantically
    identical (the engine's dispatch blocks on each wait in sequence).
    """
    import orjson

    data = orjson.loads(mybir.module_to_json_bytes(nc.m))
    ctr = 0
    for fn in data["functions"]:
        for bb in fn["blocks"]:
            out = []
            for ins in bb["instructions"]:
                si = ins.get("sync_info")
                waits = (si or {}).get("on_wait") or []
                if len(waits) > 1:
                    for w in waits[:-1]:
                        ctr += 1
                        out.append({
                            "engine": ins["engine"],
                            "ins": [],
                            "outs": [],
                            "name": f"I-sw{ctr}",
                            "opcode": "NoOp",
                            "sync_info": {"on_update": [], "on_wait": [w]},
                        })
                    si["on_wait"] = [waits[-1]]
                out.append(ins)
            bb["instructions"] = out
    nc.m = mybir.parse(data)
    return ctr


def build_mha_nc(T=2048, D=1024, DG=512, legalize=True, exp_group=2, s_bufs=2, ctx_bufs=2):
    """One-core kernel: full inputs for one (batch, head-group)."""
    HD = 64
    KC = D // P            # contraction chunks for projections (8)
    MC = DG // P           # head-pair chunks (4)
    TB = T // 512          # 512-wide t blocks (4)
    NT = T // P            # 128-wide t chunks (16)
    scale = 1.0 / np.sqrt(HD)

    nc = bass.Bass()
    xT_d = nc.declare_dram_parameter("xT", [D, T], F32R, isOutput=False)
    wq_d = nc.declare_dram_parameter("wqT", [D, DG], F32R, isOutput=False)
    wk_d = nc.declare_dram_parameter("wkT", [D, DG], F32R, isOutput=False)
    wv_d = nc.declare_dram_parameter("wvT", [D, DG], F32R, isOutput=False)
    wo_d = nc.declare_dram_parameter("woT", [DG, D], F32R, isOutput=False)
    out_d = nc.declare_dram_parameter("out", [T, D], F32, isOutput=True)

    with tile.TileContext(nc) as tc, ExitStack() as ctx:
        persist = ctx.enter_context(tc.tile_pool(name="persist", bufs=1))
        qT = persist.tile([P, MC, T], F32R, tag="qT")
        kT = persist.tile([P, MC, T], F32R, tag="kT")
        vext = persist.tile([P, NT, MC, 176], BF16, tag="vext")
        # den rows 0 (head A) and 32 (head B) receive reciprocals; partition
        # bases must be 32-aligned, so the broadcast matmul contracts over 64
        # rows with a selector that zeroes the unused ones.
        den = persist.tile([64, 512], F32R, tag="den")
        sel = persist.tile([64, P], F32R, tag="sel")
        stage_pool = tc.alloc_tile_pool(name="stage", bufs=1)
        stage_f32 = stage_pool.tile([64, 512], F32, tag="stage_f32")

        # ones columns for the denominator rows; selector rows for broadcast.
        # pad columns 65:96 / 97:128 are read as stationary junk (their output
        # rows are never consumed) but must be initialized. memset can't write
        # fp32r directly, so f32 staging + copy.
        nc.gpsimd.memset(vext[:, :, :, 64:112], 0.0)
        nc.gpsimd.memset(vext[:, :, :, 64:65], 1.0)
        nc.gpsimd.memset(vext[:, :, :, 80:81], 1.0)
        nc.gpsimd.memset(stage_f32[:], 0.0)
        nc.vector.tensor_copy(den[:], stage_f32[:])
        nc.gpsimd.memset(stage_f32[0:1, 0:64], 1.0)
        nc.gpsimd.memset(stage_f32[32:33, 64:P], 1.0)
        nc.vector.tensor_copy(sel[:], stage_f32[:, 0:P])
        stage_pool.release()

        # ---------------- phase 1: QKV projections ----------------
        with tc.tile_pool(name="wqkv", bufs=1) as wpool, \
             tc.tile_pool(name="xin", bufs=3) as xpool, \
             tc.tile_pool(name="ps_qkv", bufs=3, space="PSUM") as pq, \
             tc.tile_pool(name="ps_v", bufs=2, space="PSUM") as pvpool:
            wq_s = wpool.tile([P, KC, DG], F32R, tag="wq")
            wk_s = wpool.tile([P, KC, DG], F32R, tag="wk")
            wv_s = wpool.tile([P, KC, DG], F32R, tag="wv")
            # per-k-chunk DMAs so the first matmuls only wait for their own
            # 256KB slice, not the whole 2MB weight load. x block 0 is issued
            # before the weights so the first matmul chain starts ASAP.
            wq_r = wq_d[:].rearrange("(o p) m -> p o m", p=P)
            wk_r = wk_d[:].rearrange("(o p) m -> p o m", p=P)
            wv_r = wv_d[:].rearrange("(o p) m -> p o m", p=P)
            xT_r = xT_d[:].rearrange("(o p) t -> p o t", p=P)
            xts = []
            for tb in range(TB):
                xt_t = xpool.tile([P, KC, 512], F32R, tag="x", name=f"xt{tb}")
                xts.append(xt_t)
                for k in range(KC):
                    nc.sync.dma_start(xt_t[:, k, :],
                                      xT_r[:, k, tb * 512:(tb + 1) * 512])
                    if tb == 0:
                        nc.sync.dma_start(wq_s[:, k, :], wq_r[:, k, :])
                        nc.sync.dma_start(wk_s[:, k, :], wk_r[:, k, :])
                        nc.sync.dma_start(wv_s[:, k, :], wv_r[:, k, :])

            for tb in range(TB):
                tsl = slice(tb * 512, (tb + 1) * 512)
                xt = xts[tb]
                for tcl in range(4):
                    tg = tb * 4 + tcl
                    ps_v = pvpool.tile([P, DG], F32, tag="v")
                    for k in range(KC):
                        nc.tensor.matmul(ps_v[:], xt[:, k, tcl * P:(tcl + 1) * P],
                                         wv_s[:, k, :], start=(k == 0), stop=(k == KC - 1))
                    pv = ps_v.rearrange("p (c two e) -> p c two e", two=2, e=HD)
                    nc.vector.tensor_copy(vext[:, tg, :, 0:HD], pv[:, :, 0, :])
                    nc.vector.tensor_copy(vext[:, tg, :, 112:112 + HD], pv[:, :, 1, :])
                for c in range(MC):
                    ps_q = pq.tile([P, 512], F32, tag="q")
                    ps_k = pq.tile([P, 512], F32, tag="k")
                    for k in range(KC):
                        nc.tensor.matmul(ps_q[:], wq_s[:, k, c * P:(c + 1) * P],
                                         xt[:, k, :], start=(k == 0), stop=(k == KC - 1))
                    for k in range(KC):
                        nc.tensor.matmul(ps_k[:], wk_s[:, k, c * P:(c + 1) * P],
                                         xt[:, k, :], start=(k == 0), stop=(k == KC - 1))
                    nc.vector.tensor_copy(qT[:, c, tsl], ps_q[:])
                    nc.vector.tensor_copy(kT[:, c, tsl], ps_k[:])

        # ---------------- phases 2+3: attention, then output projection.
        # out pools allocated first (before the attention pools) so their
        # banks are disjoint from the attention pools and per-block output
        # projection overlaps attention. The wo pool opens here too so its
        # DMA overlaps the attention phase.
        ps_out = ctx.enter_context(tc.tile_pool(name="ps_out", bufs=2, space="PSUM"))
        ospool = ctx.enter_context(tc.tile_pool(name="o_sb", bufs=2))
        # ctxT is first written in the attention phase; allocating it here
        # (after the QKV pools released) frees 32KB/partition during phase 1
        # so the x pool can triple-buffer.
        ctxpool = ctx.enter_context(tc.tile_pool(name="ctx_sb", bufs=1))
        ctxT = ctxpool.tile([P, MC, T], F32R, tag="ctxT")
        wopool = ctx.enter_context(tc.tile_pool(name="wo", bufs=1))
        wo_s = wopool.tile([P, MC, D], F32R, tag="wo")
        wo_r = wo_d[:].rearrange("(o p) m -> p o m", p=P)
        for kc in range(MC):
            nc.sync.dma_start(wo_s[:, kc, :], wo_r[:, kc, :])

        # ---------------- phase 2: attention ----------------
        # PSUM: scores tag "s" [128,2,512] (2 banks) x bufs 2 = 4 banks,
        # ctx accumulators cA/cB x bufs 2 = 4 banks; the broadcast matmul
        # borrows a "s" slot. Total exactly 8 banks.
        with tc.tile_pool(name="exp", bufs=2) as epool, \
             tc.tile_pool(name="bc_sb", bufs=1) as bcpool, \
             tc.tile_pool(name="ps_s", bufs=s_bufs, space="PSUM") as pspool, \
             tc.tile_pool(name="ps_ctx", bufs=ctx_bufs, space="PSUM") as pcpool:
            for j in range(TB):
                nch = 4 * (j + 1)          # tk chunks needed for this tq block
                jsl = slice(j * 512, (j + 1) * 512)
                for c in range(MC):
                    expA = epool.tile([P, TB * 4, 512], BF16, tag="expA")
                    expB = epool.tile([P, TB * 4, 512], BF16, tag="expB")
                    ps_cA = pcpool.tile([P, 512], F32, tag="cA")
                    ps_cB = pcpool.tile([P, 512], F32, tag="cB")

                    def emit_ctx2(g2, G, c=c, expA=expA, expB=expB,
                                  ps_cA=ps_cA, ps_cB=ps_cB, nch=nch):
                        for cck in range(G * g2, G * g2 + G):
                            nc.tensor.matmul(ps_cA[:], vext[:, cck, c, 0:P],
                                             expA[:, cck, :],
                                             start=(cck == 0), stop=(cck == nch - 1))
                            nc.tensor.matmul(ps_cB[:], vext[:, cck, c, 48:176],
                                             expB[:, cck, :],
                                             start=(cck == 0), stop=(cck == nch - 1))

                    # exp_group-chunk groups: one scores psum tile per head
                    # per group, one exp instruction per head for non-diag
                    # groups. ctx matmuls run one group behind.
                    G = exp_group
                    for g in range(nch // G):
                        ck0 = G * g
                        ps_sA = pspool.tile([P, G, 512], F32, tag="s")
                        ps_sB = pspool.tile([P, G, 512], F32, tag="s")
                        for i in range(G):
                            ksl = slice((ck0 + i) * P, (ck0 + i + 1) * P)
                            nc.tensor.matmul(ps_sA[:, i, :], kT[0:64, c, ksl],
                                             qT[0:64, c, jsl], start=True, stop=True)
                            nc.tensor.matmul(ps_sB[:, i, :], kT[64:P, c, ksl],
                                             qT[64:P, c, jsl], start=True, stop=True)
                        if (ck0 + G - 1) * P - j * 512 >= 0:
                            # group overlaps the diagonal: per-chunk ranged exp
                            for i in range(G):
                                ck = ck0 + i
                                off = max(ck * P - j * 512, 0)
                                for exp_t, ps_t in ((expA, ps_sA), (expB, ps_sB)):
                                    if off > 0:
                                        nc.gpsimd.memset(exp_t[:, ck, 0:off], 0.0)
                                    nc.scalar.activation(exp_t[:, ck, off:512],
                                                         ps_t[:, i, off:512],
                                                         AF.Exp, scale=scale)
                                    if off < 512:
                                        nc.gpsimd.affine_select(
                                            out=exp_t[:, ck, off:off + P],
                                            in_=exp_t[:, ck, off:off + P],
                                            compare_op=mybir.AluOpType.is_ge,
                                            fill=0.0, base=0,
                                            pattern=[[1, P]], channel_multiplier=-1)
                        else:
                            nc.scalar.activation(expA[:, ck0:ck0 + G, :], ps_sA[:],
                                                 AF.Exp, scale=scale)
                            nc.scalar.activation(expB[:, ck0:ck0 + G, :], ps_sB[:],
                                                 AF.Exp, scale=scale)
                        if g >= 1:
                            emit_ctx2(g - 1, G)
                    emit_ctx2(nch // G - 1, G)
                    # denominators -> reciprocals -> broadcast -> normalize.
                    # den_A sits at ps_cA row 64, den_B at ps_cB row 32;
                    # 1-input DVE ops may write cross-partition-base, so both
                    # land in den rows 0/1, feeding one K=2 broadcast matmul.
                    with nc.allow_low_precision(reason="denominator reciprocal feeds an fp32r matmul; fp32r rounding is the accepted precision here"):
                        nc.vector.reciprocal(den[0:1, :], ps_cA[64:65, :])
                        nc.vector.reciprocal(den[32:33, :], ps_cB[32:33, :])
                    ps_bc_t = pspool.tile([P, exp_group, 512], F32, tag="s", name="ps_bc")
                    ps_bc = ps_bc_t[:, 0, :]
                    nc.tensor.matmul(ps_bc[:], sel[:], den[:], start=True, stop=True)
                    bc_sb = bcpool.tile([P, 512], F32, tag="bc")
                    nc.vector.tensor_copy(bc_sb[:], ps_bc[:])
                    nc.vector.tensor_mul(ctxT[0:64, c, jsl], ps_cA[0:64, :], bc_sb[0:64, :])
                    nc.vector.tensor_mul(ctxT[64:P, c, jsl], ps_cB[64:P, :], bc_sb[64:P, :])

                # output projection for this tq block — overlaps the next
                # block's attention (pre-allocated disjoint pools)
                for tcix in range(4 * j, 4 * j + 4):
                    for n in range(D // 512):
                        ps_o = ps_out.tile([P, 512], F32, tag="o")
                        for kc in range(MC):
                            nc.tensor.matmul(ps_o[:], ctxT[:, kc, tcix * P:(tcix + 1) * P],
                                             wo_s[:, kc, n * 512:(n + 1) * 512],
                                             start=(kc == 0), stop=(kc == MC - 1))
                        o_sb = ospool.tile([P, 512], F32, tag="o_sb")
                        nc.vector.tensor_copy(o_sb[:], ps_o[:])
                        nc.sync.dma_start(
                            out_d[tcix * P:(tcix + 1) * P, n * 512:(n + 1) * 512],
                            o_sb[:])
    if legalize:
        _legalize_single_wait(nc)
    return nc


_NC_CACHE = {}


def _get_nc(T=2048, D=1024, DG=512):
    key = (T, D, DG)
    if key not in _NC_CACHE:
        _NC_CACHE[key] = build_mha_nc(T, D, DG)
    return _NC_CACHE[key]


def make_in_maps(x, Wq, Wk, Wv, Wo):
    """Shard full inputs into per-core input maps (8 cores)."""
    x = np.asarray(x, dtype=np.float32)
    in_maps = []
    for core in range(8):
        b, g = core // 2, core % 2
        gs = slice(g * 512, (g + 1) * 512)
        in_maps.append({
            "xT": np.ascontiguousarray(x[b].T),
            "wqT": np.ascontiguousarray(np.asarray(Wq, np.float32)[gs, :].T),
            "wkT": np.ascontiguousarray(np.asarray(Wk, np.float32)[gs, :].T),
            "wvT": np.ascontiguousarray(np.asarray(Wv, np.float32)[gs, :].T),
            "woT": np.ascontiguousarray(np.asarray(Wo, np.float32)[:, gs].T),
        })
    return in_maps


def combine_outputs(results, bo):
    bo = np.asarray(bo, dtype=np.float32)
    outs = []
    for b in range(4):
        outs.append(results[2 * b]["out"] + results[2 * b + 1]["out"] + bo[None, :])
    return np.stack(outs, axis=0).astype(np.float32)


def run(x, Wq, Wk, Wv, Wo, bo, trace=False):
    nc = _get_nc(x.shape[1], x.shape[2], 512)
    in_maps = make_in_maps(x, Wq, Wk, Wv, Wo)
    res = run_bass_kernel_spmd(nc, in_maps, list(range(8)), trace=trace)
    return combine_outputs(res.results, bo), res


def kernel(x, Wq, Wk, Wv, Wo, bo):
    out, _ = run(x, Wq, Wk, Wv, Wo, bo)
    return out
